# revision 9
# baseline (speedup 1.0000x reference)
"""DropConnect forward kernel for Trainium2 (8 NeuronCores, Bass/Tile).

y[n,o] = (sum_k x[n,k] * weight[k,o] * w_mask[n,k,o] + bias[o]*b_mask[n,o]) * 2

Strategy: data-parallel over the batch N=256 -> 32 samples per core.
Per sample on-device:
  - DMA the sample's 4MB mask slab [1024,1024] as an SBUF tile [128, 8192]
    (contraction index k split as k = 8p + j, p = partition, j = 0..7).
  - VectorE multiplies the slab in place with a resident weight tile laid
    out identically (weight pre-scaled by 2 on host; exact in fp32).
  - TensorE reduces over k: for each j, matmul with lhsT = x[n, k(p,j)]
    column ([128,1]) accumulating into PSUM [1,512] x2 halves, dtype
    float32r (full-rate fp32 matmul path).
  - ScalarE copies PSUM rows into the per-core output tile.
Epilogue: one batched VectorE op adds 2*bias (x) b_mask, then DMA out.
"""

import sys

for _p in ("/opt/trn_rl_repo",):
    if _p not in sys.path:
        sys.path.insert(0, _p)

import numpy as np

import concourse.bass as bass
import concourse.tile as tile
from concourse import bacc, mybir
from concourse.bass_utils import run_bass_kernel_spmd

N_CORES = 8
NS = 32            # samples per core
D = 1024           # in_dim == out_dim
P = 128            # SBUF partitions
J = D // P         # 8 k-subtiles interleaved per partition row
F = J * D          # 8192 free elements per mask slab
NH = 512           # PSUM half width (one bank of fp32)

FP32 = mybir.dt.float32
FP32R = mybir.dt.float32r

# test.py pokes this to get a traced run; the grading path never touches it.
TRACE = {"trace": False, "last_result": None, "trace_kwargs": {}}


def _build_nc(ns: int = NS):
    # Bacc (not raw Bass): its compile() runs generate_event_semaphores,
    # which legalizes instructions down to <=1 semaphore wait each.
    nc = bacc.Bacc("TRN2", target_bir_lowering=False, debug=False)

    wm = nc.declare_dram_parameter("wm", [ns, P, F], FP32, isOutput=False)
    wp = nc.declare_dram_parameter("wp", [P, F], FP32, isOutput=False)
    xt = nc.declare_dram_parameter("xt", [P, J * ns], FP32, isOutput=False)
    bm = nc.declare_dram_parameter("bm", [ns, D], FP32, isOutput=False)
    b2 = nc.declare_dram_parameter("b2", [ns, D], FP32, isOutput=False)
    y = nc.declare_dram_parameter("y", [ns, D], FP32, isOutput=True)

    with tile.TileContext(nc) as tc:
        with (
            tc.tile_pool(name="const", bufs=1) as cpool,
            tc.tile_pool(name="slab", bufs=2) as spool,
            tc.tile_pool(name="prod", bufs=2) as prpool,
            tc.tile_pool(name="stage", bufs=4) as stpool,
            tc.tile_pool(name="psum", bufs=4, space=bass.MemorySpace.PSUM) as ppool,
        ):
            wpt = cpool.tile([P, F], FP32, tag="wp")
            nc.sync.dma_start(out=wpt[:], in_=wp[:])
            xtt = cpool.tile([P, J * ns], FP32, tag="xt")
            nc.sync.dma_start(out=xtt[:], in_=xt[:])
            # fp32r matmul operands must come from an op that rounds to
            # fp32r; DVE copy with an fp32r destination does that.
            xtr = cpool.tile([P, J * ns], FP32R, tag="xtr")
            nc.vector.tensor_copy(xtr[:], xtt[:])
            bmt = cpool.tile([ns, D], FP32, tag="bm")
            nc.sync.dma_start(out=bmt[:], in_=bm[:])
            b2t = cpool.tile([ns, D], FP32, tag="b2")
            nc.sync.dma_start(out=b2t[:], in_=b2[:])
            yt = cpool.tile([ns, D], FP32, tag="y")
            bbt = cpool.tile([ns, D], FP32, tag="bb")
            # bb[n,o] = 2*bias[o] * b_mask[n,o]
            nc.vector.tensor_mul(bbt[:], bmt[:], b2t[:])

            for n in range(ns):
                slab = spool.tile([P, F], FP32, tag="slab")
                nc.sync.dma_start(out=slab[:], in_=wm[n, :, :])
                # prod <- mask * (2*weight), elementwise, rounded to fp32r
                prod = prpool.tile([P, F], FP32R, tag="prod")
                nc.vector.tensor_mul(prod[:], slab[:], wpt[:])

                ps0 = ppool.tile([1, NH], FP32, tag="ps0")
                ps1 = ppool.tile([1, NH], FP32, tag="ps1")
                for j in range(J):
                    col = j * ns + n
                    lhsT = xtr[:, col : col + 1]
                    base = j * D
                    nc.tensor.matmul(
                        ps0[:],
                        lhsT,
                        prod[:, base : base + NH],
                        start=(j == 0),
                        stop=(j == J - 1),
                    )
                    nc.tensor.matmul(
                        ps1[:],
                        lhsT,
                        prod[:, base + NH : base + D],
                        start=(j == 0),
                        stop=(j == J - 1),
                    )
                # Compute engines can only address 32-aligned partition
                # offsets; bounce each sample row through a partition-0
                # staging tile, then DMA-scatter into row n of yt.
                stage = stpool.tile([1, D], FP32, tag="stage")
                nc.scalar.mul(stage[0:1, 0:NH], ps0[:], 1.0)
                nc.scalar.mul(stage[0:1, NH:D], ps1[:], 1.0)
                nc.sync.dma_start(out=yt[n : n + 1, :], in_=stage[:])

            nc.vector.tensor_add(yt[:], yt[:], bbt[:])
            nc.sync.dma_start(out=y[:], in_=yt[:])

    nc.compile()
    return nc


def _host_prep(x, weight, bias, w_mask, b_mask):
    """Shard + lay out inputs for the 8 cores. Layout-only (plus exact *2)."""
    x = np.ascontiguousarray(x, dtype=np.float32)
    weight = np.ascontiguousarray(weight, dtype=np.float32)
    bias = np.ascontiguousarray(bias, dtype=np.float32)
    b_mask = np.ascontiguousarray(b_mask, dtype=np.float32)

    wp = (2.0 * weight).reshape(P, F)                 # k = 8p + j
    b2 = np.tile((2.0 * bias)[None, :], (NS, 1)).astype(np.float32)

    in_maps = []
    for c in range(N_CORES):
        sl = slice(c * NS, (c + 1) * NS)
        wm_c = w_mask[sl].reshape(NS, P, F)           # view, no copy
        xt_c = np.ascontiguousarray(
            x[sl].T.reshape(P, J, NS).reshape(P, J * NS)
        )                                             # col = j*NS + n
        in_maps.append(
            {
                "wm": wm_c,
                "wp": wp,
                "xt": xt_c,
                "bm": np.ascontiguousarray(b_mask[sl]),
                "b2": b2,
            }
        )
    return in_maps


def kernel(x, weight, bias, w_mask, b_mask):
    in_maps = _host_prep(x, weight, bias, w_mask, b_mask)
    nc = _build_nc()
    res = run_bass_kernel_spmd(
        nc,
        in_maps,
        core_ids=list(range(N_CORES)),
        trace=TRACE["trace"],
        **TRACE["trace_kwargs"],
    )
    TRACE["last_result"] = res
    out = np.concatenate([res.results[c]["y"] for c in range(N_CORES)], axis=0)
    return out.astype(np.float32, copy=False)


# revision 14
# speedup vs baseline: 2.3003x; 2.3003x over previous
"""DropConnect forward kernel for Trainium2 (8 NeuronCores, Bass/Tile).

y[n,o] = (sum_k x[n,k] * weight[k,o] * w_mask[n,k,o] + bias[o]*b_mask[n,o]) * 2

Data-parallel over the batch N=256 -> 32 samples per core. The w_mask
values are exactly 0.0/1.0, so they are cast (losslessly) to bf16 on the
host, halving the dominant DMA traffic. Per sample on-device:
  - one 4MB DMA brings the sample's mask slab in as [128, 8192] bf16
    (contraction index k split as k = 8p + j, p = partition, j = 0..7)
  - VectorE multiplies mask x (2*weight_bf16) in 4 chunks of [128,2048]
    (bf16 tensor_tensor runs in the DVE 2x perf mode); chunking keeps
    TensorE fed at fine granularity
  - TensorE reduces over k: per j, two N=512 matmuls (lhsT = x column,
    bf16) packed CONCURRENTLY into PE column groups 0/32 via
    tile_position, accumulating into one PSUM bank (partitions 0 / 32)
  - ScalarE copies the PSUM rows to a partition-0 staging tile; a small
    DMA scatters the row into the batched output tile (compute engines
    can only address 32-aligned partitions; DMA has no such limit)
Epilogue: one batched VectorE op adds 2*bias (x) b_mask, then DMA out.
DMA ring split: mask slabs stream on the SP HWDGE ring (nc.sync);
constants/scatters/output use the ACT ring (nc.scalar) so they never
queue behind the 4MB slab transfers (HWDGE rings are FIFO per engine).

Measured on trn2 (8 cores, axon): ~196 us HW exec, rel err ~1.7e-3
(bf16 weight/x rounding; the f32 path variant measured 358 us at 1e-4).
"""

import sys

for _p in ("/opt/trn_rl_repo",):
    if _p not in sys.path:
        sys.path.insert(0, _p)

import numpy as np

import concourse.bass as bass
import concourse.tile as tile
from concourse import bacc, mybir
from concourse.bass_utils import run_bass_kernel_spmd

N_CORES = 8
NS = 32            # samples per core
D = 1024           # in_dim == out_dim
P = 128            # SBUF partitions
J = D // P         # 8 k-subtiles interleaved per partition row
F = J * D          # 8192 free elements per mask slab
NH = 512           # PSUM half width (one fp32 bank)
NCHUNK = 4         # DVE product chunks per sample
CW = F // NCHUNK   # 2048 elements per chunk (2 j-columns)

FP32 = mybir.dt.float32
FP32R = mybir.dt.float32r
BF16 = mybir.dt.bfloat16

# test.py pokes this to get a traced run; the grading path never touches it.
TRACE = {"trace": False, "last_result": None, "trace_kwargs": {}}


def _build_nc(ns: int = NS):
    # Bacc (not raw Bass): its compile() runs generate_event_semaphores,
    # which legalizes instructions down to <=1 semaphore wait each.
    nc = bacc.Bacc("TRN2", target_bir_lowering=False, debug=False)

    wm = nc.declare_dram_parameter("wm", [ns, P, F], BF16, isOutput=False)
    wp = nc.declare_dram_parameter("wp", [P, F], BF16, isOutput=False)
    xt = nc.declare_dram_parameter("xt", [P, J * ns], FP32, isOutput=False)
    bm = nc.declare_dram_parameter("bm", [ns, D], FP32, isOutput=False)
    b2 = nc.declare_dram_parameter("b2", [ns, D], FP32, isOutput=False)
    y = nc.declare_dram_parameter("y", [ns, D], FP32, isOutput=True)

    with tile.TileContext(nc) as tc:
        with (
            tc.tile_pool(name="const", bufs=1) as cpool,
            tc.tile_pool(name="slab", bufs=5) as spool,
            tc.tile_pool(name="prod", bufs=10) as prpool,
            tc.tile_pool(name="stage", bufs=4) as stpool,
            tc.tile_pool(name="psum", bufs=4, space=bass.MemorySpace.PSUM) as ppool,
        ):
            # Constants: separate chunk tiles so each DVE chunk op waits
            # only on its own 1MB weight DMA, not the whole 4MB.
            xtt = cpool.tile([P, J * ns], FP32, tag="xt")
            nc.scalar.dma_start(out=xtt[:], in_=xt[:])
            # x as bf16 for the matmul stationary operand
            xtr = cpool.tile([P, J * ns], BF16, tag="xtr")
            nc.vector.tensor_copy(xtr[:], xtt[:])
            wpt = []
            for q in range(NCHUNK):
                t = cpool.tile([P, CW], BF16, tag=f"wp{q}")
                nc.scalar.dma_start(out=t[:], in_=wp[:, q * CW : (q + 1) * CW])
                wpt.append(t)
            bmt = cpool.tile([ns, D], FP32, tag="bm")
            nc.scalar.dma_start(out=bmt[:], in_=bm[:])
            b2t = cpool.tile([ns, D], FP32, tag="b2")
            nc.scalar.dma_start(out=b2t[:], in_=b2[:])
            yt = cpool.tile([ns, D], FP32, tag="y")
            bbt = cpool.tile([ns, D], FP32, tag="bb")

            for n in range(ns):
                slab = spool.tile([P, F], BF16, tag="slab")
                nc.sync.dma_start(out=slab[:], in_=wm[n, :, :])

                ps = ppool.tile([64, NH], FP32, tag="ps")
                for q in range(NCHUNK):
                    off = q * CW
                    prod = prpool.tile([P, CW], BF16, tag="prod")
                    nc.vector.tensor_mul(
                        prod[:], slab[:, off : off + CW], wpt[q][:]
                    )
                    for jj in range(CW // D):  # 2 j-columns per chunk
                        j = q * (CW // D) + jj
                        lhsT = xtr[:, j * ns + n : j * ns + n + 1]
                        base = jj * D
                        nc.tensor.matmul(
                            ps[0:1, :],
                            lhsT,
                            prod[:, base : base + NH],
                            start=(j == 0),
                            stop=(j == J - 1),
                            tile_position=(0, 0),
                        )
                        nc.tensor.matmul(
                            ps[32:33, :],
                            lhsT,
                            prod[:, base + NH : base + D],
                            start=(j == 0),
                            stop=(j == J - 1),
                            tile_position=(0, 32),
                        )

                # Bounce each sample row through a partition-0 staging
                # tile, then DMA-scatter into row n of yt.
                stage = stpool.tile([1, D], FP32, tag="stage")
                nc.scalar.mul(stage[0:1, 0:NH], ps[0:1, :], 1.0)
                nc.scalar.mul(stage[0:1, NH:D], ps[32:33, :], 1.0)
                nc.scalar.dma_start(
                    out=yt[n : n + 1, :], in_=stage[:], single_packet=True
                )

            # bb[n,o] = 2*bias[o] * b_mask[n,o], off the critical prefix
            nc.vector.tensor_mul(bbt[:], bmt[:], b2t[:])
            nc.vector.tensor_add(yt[:], yt[:], bbt[:])
            nc.scalar.dma_start(out=y[:], in_=yt[:])

    nc.compile()
    return nc


def _host_prep(x, weight, bias, w_mask, b_mask):
    """Shard + lay out inputs for the 8 cores. Layout-only (plus exact *2)."""
    x = np.ascontiguousarray(x, dtype=np.float32)
    weight = np.ascontiguousarray(weight, dtype=np.float32)
    bias = np.ascontiguousarray(bias, dtype=np.float32)
    b_mask = np.ascontiguousarray(b_mask, dtype=np.float32)

    import ml_dtypes

    wp = (2.0 * weight).reshape(P, F).astype(ml_dtypes.bfloat16)  # k = 8p + j
    b2 = np.tile((2.0 * bias)[None, :], (NS, 1)).astype(np.float32)

    in_maps = []
    for c in range(N_CORES):
        sl = slice(c * NS, (c + 1) * NS)
        wm_c = w_mask[sl].reshape(NS, P, F).astype(ml_dtypes.bfloat16)  # exact 0/1
        xt_c = np.ascontiguousarray(
            x[sl].T.reshape(P, J, NS).reshape(P, J * NS)
        )                                             # col = j*NS + n
        in_maps.append(
            {
                "wm": wm_c,
                "wp": wp,
                "xt": xt_c,
                "bm": np.ascontiguousarray(b_mask[sl]),
                "b2": b2,
            }
        )
    return in_maps


def kernel(x, weight, bias, w_mask, b_mask):
    in_maps = _host_prep(x, weight, bias, w_mask, b_mask)
    nc = _build_nc()
    res = run_bass_kernel_spmd(
        nc,
        in_maps,
        core_ids=list(range(N_CORES)),
        trace=TRACE["trace"],
        **TRACE["trace_kwargs"],
    )
    TRACE["last_result"] = res
    out = np.concatenate([res.results[c]["y"] for c in range(N_CORES)], axis=0)
    return out.astype(np.float32, copy=False)
